# revision 50
# baseline (speedup 1.0000x reference)
"""v3 Bass program builder for the Synthesizer mixed-attention module.

Per-core math (2 heads per core, all 4 batches, tensor-parallel over heads):
  expR   = exp(rm^T)                    [t, q] layout per head
  Zr     = ones^T @ expR                (PE partition-reduce)
  ubc    = bcast(s1 / Zr)               (PE broadcast matmul)
  Lrand  = expR * ubc                   = s1 * softmax(rm)   [t, q] f16
  q_sb   = (q @ Wq + bq)^T              [hd2, b, q]  f16
  vaug   = (v @ Wv + bv | ones)         [t, 2, 65] per (b, t-chunk), f16
  a1     = relu(q_sb @ W1d + b1)        [128(2h x 64j), q]  f16 (both heads)
  lg     = w2s2[h]^T @ a1[h] + I^T @ Lrand[h]   [t, q] PSUM (rand branch
           injected into the logit PSUM by an identity matmul)
  e      = exp(lg + s2*b2)              f16 (single ACT pass, no DVE mul)
  attn   = vaug[h]^T @ e                [65, q] PSUM accumulate over t (Z row 64)
  onorm  = attn[0:64] * bcast(1/Z)      f16 (fused norm + eviction)
  out_b  = onorm^T @ Wo                 [q, o] f16 -> DRAM partial

Host sums the 8 partial outputs and adds bo.

Scheduling: per-batch software pipeline [qv(b) -> slab(b)] so input DMA for
batch b+1 prefetches during slab b; rand head 1 and outproj(b-1) are emitted
as fillers inside slab t-loops to keep PE/ACT streams dense.
"""

import sys

sys.path.insert(0, "/opt/trn_rl_repo")

from contextlib import ExitStack

import numpy as np

import concourse.bass as bass
import concourse.tile as tile
from concourse import bacc, mybir

B, S, D, H, HD = 4, 1024, 1024, 16, 64
NCORES = 8
HPC = H // NCORES  # 2
HD2 = HPC * HD  # 128
P = 128
KC = D // P  # 8
TC = S // P  # 8

F32 = mybir.dt.float32
F16 = mybir.dt.float16
BF16 = mybir.dt.bfloat16
F8 = mybir.dt.float8e4
AF = mybir.ActivationFunctionType
ALU = mybir.AluOpType
PM = mybir.MatmulPerfMode

NP_BF16 = mybir.dt.np(BF16)
NP_F8 = mybir.dt.np(F8)


def build_v3(nrep=1, inject_dve_stride=0, op_evict_act=0, norm_pool=True):
    """inject_dve_stride: every Nth rand-branch injection uses a DVE add
    (PSUM+SBUF -> SBUF) instead of a PE identity matmul (0 = all PE).
    op_evict_act: of each qc's 2 outproj evictions, how many go to ACT.
    norm_pool: run the 1/Z normalization multiplies on GpSimd (SBUF-only).
    """
    nc = bacc.Bacc("TRN2", target_bir_lowering=False, debug=False, num_devices=NCORES)

    qT = nc.dram_tensor("qT", [B, D, S], F8, kind="ExternalInput")
    vT = nc.dram_tensor("vT", [B, D, S], F16, kind="ExternalInput")
    rmT = nc.dram_tensor("rmT", [HPC, S, S], F16, kind="ExternalInput")
    wq = nc.dram_tensor("wq", [D, HD2], F8, kind="ExternalInput")
    wv = nc.dram_tensor("wv", [D, HD2], F16, kind="ExternalInput")
    w1d = nc.dram_tensor("w1d", [P, HD], F16, kind="ExternalInput")  # [W1;W1]
    # W2 in DoubleRow k-tile layout: [r 32, jt 2, S] dup'd for 2 heads
    w2f = nc.dram_tensor("w2f", [HD, 2, S], F16, kind="ExternalInput")
    wo = nc.dram_tensor("wo", [HD2, D], F16, kind="ExternalInput")
    bq = nc.dram_tensor("bq", [HD2, 1], F32, kind="ExternalInput")
    bv_row = nc.dram_tensor("bv_row", [1, HD2], F16, kind="ExternalInput")
    b1f = nc.dram_tensor("b1f", [HD, 2], F32, kind="ExternalInput")  # folded b1
    b2c = nc.dram_tensor("b2c", [P, TC], F32, kind="ExternalInput")
    alpha = nc.dram_tensor("alpha", [1, 2], F32, kind="ExternalInput")
    ident = nc.dram_tensor("ident", [P, P], F16, kind="ExternalInput")
    out = nc.dram_tensor("out", [B, S, D], F16, kind="ExternalOutput")

    with tile.TileContext(nc) as tc, ExitStack() as ctx:
        ctx.enter_context(
            nc.allow_low_precision(
                reason="16-bit softmax/normalization intermediates are within "
                "the 2e-2 relative error budget"
            )
        )
        consts = ctx.enter_context(tc.tile_pool(name="consts", bufs=1))
        persist = ctx.enter_context(tc.tile_pool(name="persist", bufs=1))
        qstage = ctx.enter_context(tc.tile_pool(name="qstage", bufs=2))
        vstage = ctx.enter_context(tc.tile_pool(name="vstage", bufs=2))
        rmstage = ctx.enter_context(tc.tile_pool(name="rmstage", bufs=6))
        ework = ctx.enter_context(tc.tile_pool(name="ework", bufs=3))
        a1pool = ctx.enter_context(tc.tile_pool(name="a1pool", bufs=2))
        obpool = ctx.enter_context(tc.tile_pool(name="obpool", bufs=3))
        small = ctx.enter_context(tc.tile_pool(name="small", bufs=2))
        onpool = ctx.enter_context(tc.tile_pool(name="onpool", bufs=1))
        nscr = ctx.enter_context(tc.tile_pool(name="nscr", bufs=3))
        dscr = ctx.enter_context(tc.tile_pool(name="dscr", bufs=2, space="DRAM"))
        # PSUM banks: ps_big 2x2 + ps_at 1x2 + ps_sm 2x1 = 8
        ps_big = ctx.enter_context(tc.tile_pool(name="ps_big", bufs=2, space="PSUM"))
        ps_at = ctx.enter_context(tc.tile_pool(name="ps_at", bufs=1, space="PSUM"))
        ps_sm = ctx.enter_context(tc.tile_pool(name="ps_sm", bufs=2, space="PSUM"))

        # ---- early consts (tiny: scalars + ones) --------------------------
        al = consts.tile([P, 2], F32, tag="al")
        nc.scalar.dma_start(al[:], alpha[:].to_broadcast((P, 2)))
        denom = consts.tile([P, 1], F32, tag="denom")
        nc.vector.tensor_add(denom[:], al[:, 0:1], al[:, 1:2])
        rden = consts.tile([P, 1], F32, tag="rden")
        nc.vector.reciprocal(rden[:], denom[:])
        s1 = consts.tile([P, 1], F32, tag="s1")
        nc.vector.tensor_mul(s1[:], al[:, 0:1], rden[:])
        s2bc = consts.tile([P, 1], F32, tag="s2bc")
        nc.vector.tensor_mul(s2bc[:], al[:, 1:2], rden[:])
        # logits are computed x32 in PSUM (keeps fp8 W2 out of subnormals);
        # the e-exp applies scale=1/32
        s2bc32 = consts.tile([P, 1], F32, tag="s2bc32")
        nc.vector.tensor_scalar_mul(s2bc32[:], s2bc[:], 32.0)
        ones_col = consts.tile([P, 1], F16, tag="onescol")
        nc.vector.memset(ones_col[:], 1.0)
        ones_f16 = consts.tile([1, P], F16, tag="onesf16")
        nc.vector.memset(ones_f16[:], 1.0)
        s1_32 = consts.tile([P, 1], F32, tag="s1_32")
        nc.vector.tensor_scalar_mul(s1_32[:], s1[:], 32.0)
        s1row = consts.tile([1, P], F16, tag="s1row")
        nc.vector.tensor_scalar_mul(s1row[:], ones_f16[:], s1_32[0:1, :])

        # ---- weights/biases: emitted AFTER the first staging DMAs so the
        # rm/q/v loads head the DMA pipe (weights aren't needed until the
        # first projections). Filled into W by emit_weights().
        W = {}

        def emit_weights():
            W["wq_t"] = consts.tile([P, KC, HD2], F8, tag="wq", name="wq_t")
            nc.scalar.dma_start(
                W["wq_t"][:], wq[:].rearrange("(c p) m -> p c m", p=P)
            )
            W["wv_t"] = consts.tile([P, KC, HD2], F16, tag="wv", name="wv_t")
            nc.scalar.dma_start(
                W["wv_t"][:], wv[:].rearrange("(c p) m -> p c m", p=P)
            )
            W["w1d_t"] = consts.tile([P, HD], F16, tag="w1d", name="w1d_t")
            nc.scalar.dma_start(W["w1d_t"][:], w1d[:])
            w2f_ld = consts.tile([HD, 2, S], F16, tag="w2ld")
            nc.scalar.dma_start(w2f_ld[:], w2f[:])
            # fp8 DoubleRow W2 (x32 scaled: values land in fp8 normal range)
            w2s28_t = consts.tile([HD, 2, S], F8, tag="w2s8")
            nc.scalar.activation(
                w2s28_t[:], w2f_ld[:], AF.Copy, scale=s2bc32[0:HD, :]
            )
            W["w2s28"] = w2s28_t
            W["wo_t"] = consts.tile([HD2, D], F16, tag="wo", name="wo_t")
            nc.scalar.dma_start(W["wo_t"][:], wo[:])
            W["ident_t"] = consts.tile([P, P], F16, tag="ident", name="ident_t")
            nc.scalar.dma_start(W["ident_t"][:], ident[:])
            bcat_ld = consts.tile([P, 1], F32, tag="bcatld")
            nc.scalar.dma_start(bcat_ld[:], bq[:])
            W["b1f_t"] = consts.tile([HD, 2], F32, tag="b1f", name="b1f_t")
            nc.scalar.dma_start(W["b1f_t"][:], b1f[:])
            b2_ld = consts.tile([P, TC], F32, tag="b2ld")
            nc.scalar.dma_start(b2_ld[:], b2c[:])
            bcat = consts.tile([P, 1], F32, tag="bcat")
            nc.vector.tensor_copy(bcat[:], bcat_ld[:])
            W["bq_t"] = bcat[:, 0:1]
            W["b2s"] = consts.tile([P, TC], F32, tag="b2s", name="b2s")
            nc.vector.tensor_tensor(
                W["b2s"][:], b2_ld[:], s2bc[:].to_broadcast((P, TC)), ALU.mult
            )
            W["bvr_t"] = consts.tile([1, HD2], F16, tag="bvr", name="bvr_t")
            nc.scalar.dma_start(W["bvr_t"][:], bv_row[:])

        def make_rep(rep):
            lrand = {}
            vaug = {}
            onorm = {}
            box = {}
            injctr = [0]

            def emit_rm_dma(h, state):
                """Issue the rm staging DMAs for head h (prefetch only)."""
                rmts = []
                for tp in range(TC // 2):
                    rmt = rmstage.tile([P, 2, S], F16, tag="rmt", name=f"rm{h}_{tp}")
                    nc.sync.dma_start(
                        rmt[:],
                        rmT[h, tp * 2 * P : (tp + 1) * 2 * P, :].rearrange(
                            "(c p) s -> p c s", p=P
                        ),
                    )
                    rmts.append(rmt)
                state["rmts"] = rmts

            def emit_rand_part1(h, state):
                """expR exp + Zr accumulation (issues rm DMA if not yet)."""
                if "rmts" not in state:
                    emit_rm_dma(h, state)
                expR = persist.tile([P, TC, S], F16, tag="expR", name=f"expR{h}")
                zr = [
                    ps_sm.tile([1, 512], F32, tag="sm", name=f"zr{h}_{q}")
                    for q in range(2)
                ]
                for tp in range(TC // 2):
                    rmt = state["rmts"][tp]
                    nc.scalar.activation(
                        expR[:, tp * 2 : tp * 2 + 2, :], rmt[:], AF.Exp
                    )
                    for tt in range(2):
                        t = tp * 2 + tt
                        for q in range(2):
                            nc.tensor.matmul(
                                zr[q][:],
                                lhsT=ones_col[:],
                                rhs=expR[:, t, q * 512 : (q + 1) * 512],
                                start=(t == 0),
                                stop=(t == TC - 1),
                            )
                state["expR"] = expR
                state["zr"] = zr

            def emit_rand_part2(h, state):
                """1/Zr, ubc broadcast, Lrand = expR * ubc (= s1*softmax(rm))."""
                expR, zr = state["expR"], state["zr"]
                lrand[h] = persist.tile([P, TC, S], F16, tag=f"Lr{h}", name=f"Lr{h}")
                rz = small.tile([1, S], F16, tag="rz")
                for q in range(2):
                    nc.vector.reciprocal(rz[:, q * 512 : (q + 1) * 512], zr[q][:])
                ubc = small.tile([P, S], F16, tag="ubc", name=f"ubc{h}")
                for q in range(2):
                    ub_ps = ps_sm.tile([P, 512], F32, tag="sm", name=f"ub{h}_{q}")
                    nc.tensor.matmul(
                        ub_ps[:],
                        lhsT=s1row[:],
                        rhs=rz[:, q * 512 : (q + 1) * 512],
                        start=True,
                        stop=True,
                    )
                    nc.vector.tensor_copy(ubc[:, q * 512 : (q + 1) * 512], ub_ps[:])
                for t in range(TC):
                    nc.vector.tensor_mul(
                        lrand[h][:, t, :], expR[:, t, :], ubc[:]
                    )

            qstgs = {}
            vstgs = {}

            def emit_qv_dma(b):
                """Issue q/v staging DMAs for batch b (prefetch, no compute)."""
                qstg = qstage.tile([P, KC, S], F8, tag="qstg", name=f"q{b}")
                nc.sync.dma_start(
                    qstg[:], qT[b].rearrange("(c p) s -> p c s", p=P)
                )
                vstg = vstage.tile([P, KC, S], F16, tag="vstg", name=f"v{b}")
                nc.sync.dma_start(
                    vstg[:], vT[b].rearrange("(c p) s -> p c s", p=P)
                )
                qstgs[b] = qstg
                vstgs[b] = vstg

            def emit_qproj(b):
                if "qT_sb" not in box:
                    box["qT_sb"] = persist.tile(
                        [HD2, B, S], F16, tag="qTsb", name="qTsb"
                    )
                qT_sb = box["qT_sb"]
                q_ps = [
                    ps_sm.tile([HD2, 512], F32, tag="sm", name=f"qps{b}_{q}")
                    for q in range(2)
                ]
                # fp8 DoubleRow: contraction over d as 4 k-tile pairs
                for c2 in range(KC // 2):
                    for q in range(2):
                        sl = slice(q * 512, (q + 1) * 512)
                        nc.tensor.matmul(
                            q_ps[q][:],
                            lhsT=W["wq_t"][:, 2 * c2 : 2 * c2 + 2, :],
                            rhs=qstgs[b][:, 2 * c2 : 2 * c2 + 2, sl],
                            start=(c2 == 0),
                            stop=(c2 == KC // 2 - 1),
                            perf_mode=PM.DoubleRow,
                        )
                for q in range(2):
                    nc.scalar.activation(
                        qT_sb[:, b, q * 512 : (q + 1) * 512],
                        q_ps[q][:],
                        AF.Identity,
                        bias=W["bq_t"],
                    )

            def emit_vproj(b, trange=None):
                if trange is None or trange[0] == 0:
                    va = persist.tile(
                        [P, TC, 2, HD + 1], F16, tag=f"vaug{b}", name=f"vaug{b}"
                    )
                    vaug[b] = va
                    nc.vector.memset(va[:, :, :, HD : HD + 1], 1.0)
                va = vaug[b]
                for t in range(*(trange or (0, TC))):
                    vp = ps_big.tile([P, 2, HD], F32, tag="big", name=f"vp{b}_{t}")
                    for kc in range(KC):
                        nc.tensor.matmul(
                            vp[:],
                            lhsT=vstgs[b][:, kc, t * P : (t + 1) * P],
                            rhs=W["wv_t"][:, kc, :],
                            start=(kc == 0),
                            stop=False,
                        )
                    nc.tensor.matmul(
                        vp[:], lhsT=ones_f16[:], rhs=W["bvr_t"][:], start=False, stop=True
                    )
                    nc.vector.tensor_copy(va[:, t, :, 0:HD], vp[:])

            a1s = {}

            def emit_dense1(b):
                """a1 = relu(q W1 + b1) evicted to fp8 in DoubleRow k-tile
                layout [64p (2h x 32r), 2jt, q]; jt = j//32 along free so the
                lg matmul can run fp8 DoubleRow with K = (32p, 2jt)."""
                qT_sb = box["qT_sb"]
                onorm[b] = onpool.tile(
                    [HD2, S], F16, tag=f"onorm{b}", name=f"onorm{b}"
                )
                a18 = a1pool.tile([HD, 2, S], F8, tag="a1", name=f"a1{b}")
                for q in range(2):
                    sl = slice(q * 512, (q + 1) * 512)
                    aq = [
                        ps_sm.tile([HD, 512], F32, tag="sm", name=f"a1ps{b}_{q}{jt}")
                        for jt in range(2)
                    ]
                    for h in range(2):
                        hs = slice(h * HD, (h + 1) * HD)
                        for jt in range(2):
                            nc.tensor.matmul(
                                aq[jt][h * 32 : (h + 1) * 32, :],
                                lhsT=W["w1d_t"][hs, jt * 32 : (jt + 1) * 32],
                                rhs=qT_sb[hs, b, sl],
                                start=True,
                                stop=True,
                            )
                    for jt in range(2):
                        if q == 0:
                            nc.scalar.activation(
                                a18[:, jt, sl],
                                aq[jt][:],
                                AF.Relu,
                                bias=W["b1f_t"][:, jt : jt + 1],
                            )
                        else:
                            nc.vector.tensor_scalar(
                                a18[:, jt, sl],
                                aq[jt][:],
                                W["b1f_t"][:, jt : jt + 1],
                                0.0,
                                ALU.add,
                                ALU.max,
                            )
                a1s[b] = a18
                return a18

            def outproj_chunks(b):
                """Output projection for batch b as 4 filler closures."""

                def mk(qc0):
                    def f():
                        emit_outproj(b, (qc0, qc0 + 1))

                    return f

                return [mk(0), mk(2), mk(4), mk(6)]

            def emit_outproj(b, qcs=tuple(range(TC)), spread=False):
                """Output projection for batch b; evictions DVE (or ACT/DVE
                alternating when spread=True, for the unoverlapped tail)."""
                for qc in qcs:
                    ob = obpool.tile([P, D], F16, tag="ob")
                    for oc in range(2):
                        sl = slice(oc * 512, (oc + 1) * 512)
                        op_ps = ps_sm.tile(
                            [P, 512], F32, tag="sm", name=f"op{b}{qc}{oc}"
                        )
                        nc.tensor.matmul(
                            op_ps[:],
                            lhsT=onorm[b][:, qc * P : (qc + 1) * P],
                            rhs=W["wo_t"][:, sl],
                            start=True,
                            stop=True,
                        )
                        act_evict = op_evict_act if not spread else 1
                        if oc < act_evict:
                            nc.scalar.activation(ob[:, sl], op_ps[:], AF.Copy)
                        else:
                            nc.vector.tensor_copy(ob[:, sl], op_ps[:])
                    nc.sync.dma_start(out[b, qc * P : (qc + 1) * P, :], ob[:])

            def emit_slab(b, h, a1_sb, fillers, last=False):
                hs = slice(h * HD, (h + 1) * HD)
                h32 = slice(h * 32, (h + 1) * 32)
                attn_ps = ps_at.tile([HD + 1, S], F32, tag="at")
                es = {}

                def emit_logits(t):
                    lg = ps_big.tile([P, S], F32, tag="big", name=f"lg{b}{h}{t}")
                    injctr[0] += 1
                    inj_dve = bool(
                        inject_dve_stride and injctr[0] % inject_dve_stride == 0
                    )
                    for q in range(2):
                        sl = slice(q * 512, (q + 1) * 512)
                        # dense logits: fp8 DoubleRow, K = (32p, 2jt) = 64
                        nc.tensor.matmul(
                            lg[:, sl],
                            lhsT=W["w2s28"][h32, :, t * P : (t + 1) * P],
                            rhs=a1_sb[h32, :, sl],
                            start=True,
                            stop=inj_dve,
                            perf_mode=PM.DoubleRow,
                        )
                        if not inj_dve:
                            nc.tensor.matmul(
                                lg[:, sl],
                                lhsT=W["ident_t"][:],
                                rhs=lrand[h][:, t, sl],
                                start=False,
                                stop=True,
                            )
                    e_t = ework.tile([P, S], F16, tag="E")
                    if inj_dve:
                        # rand-branch inject on DVE: e_in = lg + Lrand (SBUF),
                        # then exp from SBUF on ACT
                        e_in = ework.tile([P, S], F16, tag="EIN")
                        nc.vector.tensor_add(e_in[:], lg[:], lrand[h][:, t, :])
                        nc.scalar.activation(
                            e_t[:],
                            e_in[:],
                            AF.Exp,
                            bias=W["b2s"][:, t : t + 1],
                            scale=1.0 / 32.0,
                        )
                    else:
                        nc.scalar.activation(
                            e_t[:],
                            lg[:],
                            AF.Exp,
                            bias=W["b2s"][:, t : t + 1],
                            scale=1.0 / 32.0,
                        )
                    es[t] = e_t

                def emit_attn(t):
                    e_t = es.pop(t)
                    for q in range(2):
                        sl = slice(q * 512, (q + 1) * 512)
                        nc.tensor.matmul(
                            attn_ps[:, sl],
                            lhsT=vaug[b][:, t, h, :],
                            rhs=e_t[:, sl],
                            start=(t == 0),
                            stop=(t == TC - 1),
                        )

                # software-pipelined: attn(t-1) is emitted after logits(t),
                # so the PE never sits waiting on exp(t) right after lg(t)
                emit_logits(0)
                for t in range(1, TC):
                    emit_logits(t)
                    emit_attn(t - 1)
                    if fillers and t in (1, 3, 5, 7):
                        fillers.pop(0)()
                emit_attn(TC - 1)
                # evict unnormalized attention immediately (frees the 1-deep
                # attn PSUM for the next slab ~4us earlier), then normalize
                # off the critical path (1/Z broadcast via DRAM round-trip)
                unn = nscr.tile([HD + 1, S], F16, tag="unn")
                # split eviction across ACT+DVE: halves run in parallel, so
                # the ps_at WAR for the next slab's first attn matmul clears
                # in ~half the time
                nc.scalar.activation(unn[:, 0:512], attn_ps[:, 0:512], AF.Copy)
                nc.vector.tensor_copy(unn[:, 512:1024], attn_ps[:, 512:1024])
                rzq = nscr.tile([1, S], F16, tag="rzq")
                nc.vector.reciprocal(rzq[:], unn[HD : HD + 1, :])
                if last:
                    # tail: latency matters and PE is idle -- broadcast 1/Z
                    # via an outer-product matmul instead of the DRAM trip
                    rzb_ps = ps_at.tile([HD, S], F32, tag="at", name="rzbps")
                    for q in range(2):
                        sl = slice(q * 512, (q + 1) * 512)
                        nc.tensor.matmul(
                            rzb_ps[:, sl],
                            lhsT=ones_f16[:, 0:HD],
                            rhs=rzq[:, sl],
                            start=True,
                            stop=True,
                        )
                    nc.vector.tensor_mul(onorm[b][hs, :], unn[0:HD, :], rzb_ps[:])
                else:
                    rz_d = dscr.tile([1, S], F16, tag="rzd")
                    nc.scalar.dma_start(rz_d[:], rzq[:])
                    rzb_sb = nscr.tile([HD, S], F16, tag="rzb")
                    nc.scalar.dma_start(rzb_sb[:], rz_d[:].to_broadcast((HD, S)))
                    eng = nc.gpsimd if norm_pool else nc.vector
                    eng.tensor_mul(onorm[b][hs, :], unn[0:HD, :], rzb_sb[:])

            # expose emitters for the cross-rep scheduler
            return {
                "a1s": a1s,
                "rm_dma": emit_rm_dma,
                "rand1": emit_rand_part1,
                "rand2": emit_rand_part2,
                "qv_dma": emit_qv_dma,
                "qproj": emit_qproj,
                "vproj": emit_vproj,
                "dense1": emit_dense1,
                "outproj": emit_outproj,
                "outproj_chunks": outproj_chunks,
                "slab": emit_slab,
            }

        # ---- cross-rep pipelined schedule --------------------------------
        # boundary between batches carries only dense1; projections for b+1
        # run as fillers inside slab(b,1) (their DMAs issue in slab(b,0)),
        # and rep r+1's prologue + b0 projections overlap rep r's tail.
        reps = [make_rep(r) for r in range(nrep)]
        rstates = [{"rs0": {}, "rs1": {}} for _ in range(nrep)]
        for r in range(nrep):
            cur = reps[r]
            rs0, rs1 = rstates[r]["rs0"], rstates[r]["rs1"]
            if r == 0:
                cur["rand1"](0, rs0)
                cur["qv_dma"](0)
                emit_weights()
                cur["rand2"](0, rs0)
                cur["qproj"](0)
                cur["vproj"](0)
                cur["rm_dma"](1, rs1)
            # for r > 0 the prologue and b0 projections were pre-emitted
            # inside rep r-1's last slab / tail
            nxt = reps[r + 1] if r < nrep - 1 else None
            ns0 = rstates[r + 1]["rs0"] if nxt is not None else None
            ns1 = rstates[r + 1]["rs1"] if nxt is not None else None
            for b in range(B):
                a1_sb = cur["a1s"].get(b) or cur["dense1"](b)
                if b == 0:
                    f0 = [
                        lambda: (cur["rand1"](1, rs1), cur["qv_dma"](1))[0],
                        lambda: cur["rand2"](1, rs1),
                    ]
                else:
                    f0 = cur["outproj_chunks"](b - 1)
                    chunk0 = f0[0]
                    if b < B - 1:
                        f0[0] = lambda bb=b, c0=chunk0: (cur["qv_dma"](bb + 1), c0())[1]
                    elif nxt is not None:
                        # next rep's rm(h0) staging DMA gets a slab of lead
                        f0[0] = lambda c0=chunk0: (nxt["rm_dma"](0, ns0), c0())[1]
                cur["slab"](b, 0, a1_sb, f0)
                if b < B - 1:
                    f1 = [
                        lambda bb=b: cur["qproj"](bb + 1),
                        lambda bb=b: cur["vproj"](bb + 1, (0, TC // 2)),
                        lambda bb=b: cur["vproj"](bb + 1, (TC // 2, TC)),
                        lambda bb=b: cur["dense1"](bb + 1),
                    ]
                    if b == B - 2 and nxt is not None:
                        # next rep's q/v staging DMAs: two slabs of lead time
                        q0 = f1[0]
                        f1[0] = lambda c0=q0: (nxt["qv_dma"](0), c0())[1]
                elif nxt is not None:
                    f1 = [
                        lambda: nxt["rand1"](0, ns0),
                        lambda: nxt["rand2"](0, ns0),
                        lambda: nxt["rm_dma"](1, ns1),
                    ]
                else:
                    f1 = []
                cur["slab"](b, 1, a1_sb, f1, last=(b == B - 1))
            # next-rep projections BEFORE the last outproj: they don't depend
            # on onorm, so they keep PE busy during the final normalization
            if r < nrep - 1:
                nxt = reps[r + 1]
                nxt["qproj"](0)
                nxt["vproj"](0)
                nxt["dense1"](0)
            cur["outproj"](B - 1, spread=(r == nrep - 1))
    nc.finalize()
    return nc


def make_in_maps_v2(inputs):
    f32 = lambda x: np.asarray(x, np.float32)
    query = f32(inputs["query"])
    value = f32(inputs["value"])
    Wq = f32(inputs["Wq"])
    Wv = f32(inputs["Wv"])
    W1 = f32(inputs["W1"])
    W2 = f32(inputs["W2"])
    Wo = f32(inputs["Wo"])
    bq = f32(inputs["bq"])
    bv = f32(inputs["bv"])
    b1 = f32(inputs["b1"])
    b2 = f32(inputs["b2"])
    rm = f32(inputs["random_mat"])
    a1 = f32(inputs["alpha_one"])
    a2 = f32(inputs["alpha_two"])

    qTn = np.ascontiguousarray(query.transpose(0, 2, 1)).astype(NP_F8)
    vTn = np.ascontiguousarray(value.transpose(0, 2, 1)).astype(np.float16)
    w1dn = np.concatenate([W1, W1], axis=0).astype(np.float16)
    # DoubleRow k-tile fold: j = jt*32 + r -> [r, jt, t], dup'd per head
    w2fold = np.ascontiguousarray(
        W2.reshape(2, 32, S).transpose(1, 0, 2)
    ).astype(np.float16)
    w2fn = np.concatenate([w2fold, w2fold], axis=0)  # [64, 2, S]
    b1fold = b1.reshape(2, 32).T.astype(np.float32)  # [32, 2]
    b1fn = np.concatenate([b1fold, b1fold], axis=0)  # [64, 2]
    b2cn = np.ascontiguousarray(b2.reshape(TC, P).T)
    alpha = np.array([[a1[0], a2[0]]], np.float32)
    identn = np.eye(P, dtype=np.float16)

    in_maps = []
    for c in range(NCORES):
        h0 = c * HPC
        in_maps.append(
            {
                "qT": qTn,
                "vT": vTn,
                "rmT": np.ascontiguousarray(
                    rm[h0 : h0 + HPC].transpose(0, 2, 1)
                ).astype(np.float16),
                "wq": np.ascontiguousarray(
                    Wq[:, h0 : h0 + HPC, :].reshape(D, HD2)
                ).astype(NP_F8),
                "wv": np.ascontiguousarray(
                    Wv[:, h0 : h0 + HPC, :].reshape(D, HD2)
                ).astype(np.float16),
                "w1d": w1dn,
                "w2f": w2fn,
                "wo": np.ascontiguousarray(Wo[h0 : h0 + HPC].reshape(HD2, D)).astype(
                    np.float16
                ),
                "bq": np.ascontiguousarray(bq[h0 : h0 + HPC].reshape(HD2, 1)),
                "bv_row": np.ascontiguousarray(
                    bv[h0 : h0 + HPC].reshape(1, HD2)
                ).astype(np.float16),
                "b1f": b1fn,
                "b2c": b2cn,
                "alpha": alpha,
                "ident": identn,
            }
        )
    return in_maps


_CACHE = {}


def _get_program(nrep=1):
    if nrep not in _CACHE:
        _CACHE[nrep] = build_v3(nrep=nrep)
    return _CACHE[nrep]


def run(inputs, trace=False):
    """Run the SPMD kernel; returns (output, BassKernelResults)."""
    from concourse.bass_utils import run_bass_kernel_spmd

    nc = _get_program(1)
    in_maps = make_in_maps_v2(inputs)
    try:
        res = run_bass_kernel_spmd(nc, in_maps, list(range(NCORES)), trace=trace)
    except Exception:
        # a previously-crashed process can leave a NeuronCore wedged
        # (NRT_EXEC_UNIT_UNRECOVERABLE); one retry reliably clears it
        res = run_bass_kernel_spmd(nc, in_maps, list(range(NCORES)), trace=trace)
    bo = np.asarray(inputs["bo"], np.float32)
    acc = np.zeros((B, S, D), np.float32)
    for c in range(NCORES):
        acc += res.results[c]["out"].astype(np.float32)
    acc += bo[None, None, :]
    return acc, res


def kernel(**inputs) -> np.ndarray:
    out, _ = run(inputs, trace=False)
    return out
